# revision 33
# baseline (speedup 1.0000x reference)
"""Histogram-binning (AdaptiveAvgPoolSequence) Trainium2 kernel.

Full inputs -> shard points N across 8 NeuronCores -> per-core one-hot
matmul accumulation of per-bin sums+counts -> AllReduce -> divide ->
gather output.

Self-contained: hardcodes shapes from the problem spec.
"""

import sys

import numpy as np

sys.path.insert(0, "/opt/trn_rl_repo")

from concourse import bacc, bass, mybir  # noqa: E402
import concourse.tile as tile  # noqa: E402
from concourse.bass_utils import run_bass_kernel_spmd  # noqa: E402

P = 128  # partitions / points per matmul tile
K = 256  # bins (16 x 16)
H = 16
EPS = 1e-6
N_CORES = 8
MAGIC = 12582912.0  # 1.5 * 2**23: float32 round-to-int trick
SCALE = 16.0 / (2.0 + 2.0 * EPS)
OFF = (1.0 + EPS) * SCALE


def group_plan(nloc: int, g: int):
    """DMA group sizes (point-tiles each): g-sized groups with a tapered
    tail so the last group's convert/one-hot/matmul chain is short."""
    nt = nloc // P
    taper = [t for t in (g // 2, g // 4, g // 8, g // 8) if t >= 1]
    while sum(taper) + g > nt:  # tiny problems: no taper
        taper = taper[1:] if taper else []
    body = (nt - sum(taper)) // g
    plan = [g] * body + taper
    rem = nt - sum(plan)
    assert rem == 0, (nt, plan)
    return plan


def build_nc(nloc: int, g: int, bc: int, n_cores: int = N_CORES):
    """Build the per-core Bass graph.

    nloc: points per core; g: point-tiles per DMA group (also the number of
    consecutive DRAM rows each partition loads contiguously); bc: B*C.

    Data layout: group a (size ga) covers points [base, base+ga*P). Partition
    p holds DRAM rows base + p*ga .. + ga (a contiguous ga*bc*4-byte run),
    i.e. point id = base + p*ga + q -> (partition p, sub-tile q). Host-side
    coords packing must use the same order.
    """
    f = bc + 1  # +1 ones column -> counts
    nt = nloc // P
    plan = group_plan(nloc, g)
    assert nloc % P == 0

    nper = P // n_cores  # bins-rows per core after ReduceScatter
    nc = bacc.Bacc(
        "TRN2",
        target_bir_lowering=False,
        debug=False,
        num_devices=n_cores,
    )
    data = nc.declare_dram_parameter("data", [nloc, bc], mybir.dt.float32, isOutput=False)
    xyp = nc.declare_dram_parameter("xyp", [P, 2 * nt], mybir.dt.float32, isOutput=False)
    iota = nc.declare_dram_parameter("iota", [P, K], mybir.dt.float32, isOutput=False)
    out = nc.declare_dram_parameter("out", [nper, 2 * f], mybir.dt.float32, isOutput=True)

    add = mybir.AluOpType.add
    sub = mybir.AluOpType.subtract
    mult = mybir.AluOpType.mult
    iseq = mybir.AluOpType.is_equal
    bf16 = mybir.dt.bfloat16
    f32 = mybir.dt.float32

    with tile.TileContext(nc) as tc:
        with (
            tc.tile_pool(name="const", bufs=1) as constp,
            tc.tile_pool(name="coord", bufs=1) as coordp,
            tc.tile_pool(name="dfp", bufs=6) as dpool,
            tc.tile_pool(name="dbf", bufs=4) as bpool,
            tc.tile_pool(name="oh", bufs=3) as ohpool,
            tc.tile_pool(name="fin", bufs=1) as finp,
            tc.tile_pool(name="ps", bufs=1, space="PSUM") as psump,
            tc.tile_pool(name="dram", bufs=1, space="DRAM") as dramp,
        ):
            # iota constant: row 0..K-1 per partition, f32 + bf16 copies
            iota_f = constp.tile([P, K], f32)
            nc.sync.dma_start(out=iota_f[:], in_=iota[:])
            iota_b = constp.tile([P, K], bf16)
            nc.vector.tensor_copy(out=iota_b[:], in_=iota_f[:])

            # Tiny warm-up collective up front: wakes the ncfw/TOPSP path so
            # the real ReduceScatter at the end skips the ~11.5us cold
            # trigger-to-start delay.
            warm_in = dramp.tile([P, 4], f32)
            warm_out = dramp.tile([nper, 4], f32)
            warm_sb = constp.tile([P, 4], f32)
            nc.vector.tensor_copy(out=warm_sb[:], in_=iota_f[:, 0:4])
            nc.sync.dma_start(out=warm_in[:], in_=warm_sb[:])
            nc.gpsimd.collective_compute(
                "ReduceScatter",
                mybir.AluOpType.add,
                replica_groups=[list(range(n_cores))],
                ins=[warm_in[:].opt()],
                outs=[warm_out[:].opt()],
            )

            # coords -> per-tile bin column. xyp[:, t] = x of point t*P+p,
            # xyp[:, nt+t] = y.
            xy = coordp.tile([P, 2 * nt], f32)
            nc.sync.dma_start(out=xy[:], in_=xyp[:])
            # v = x*SCALE + (OFF - 0.5); k = (v + MAGIC) - MAGIC  (= round(v))
            v = coordp.tile([P, 2 * nt], f32)
            nc.vector.tensor_scalar(
                out=v[:], in0=xy[:], scalar1=SCALE, scalar2=OFF - 0.5, op0=mult, op1=add
            )
            kxy = coordp.tile([P, 2 * nt], f32)
            nc.vector.tensor_scalar(
                out=kxy[:], in0=v[:], scalar1=MAGIC, scalar2=MAGIC, op0=add, op1=sub
            )
            # bins = kx + 16*ky (bf16: exact for integers < 256)
            bins = coordp.tile([P, nt], bf16)
            nc.vector.scalar_tensor_tensor(
                out=bins[:], in0=kxy[:, nt:], scalar=16.0, in1=kxy[:, 0:nt],
                op0=mult, op1=add,
            )

            ps0 = psump.tile([P, f], mybir.dt.float32)
            ps1 = psump.tile([P, f], mybir.dt.float32)

            base = 0  # point offset of current group
            bt = 0  # tile offset of current group
            for a, ga in enumerate(plan):
                dt32 = dpool.tile([P, g * bc], f32, tag="dt32")
                data_src = data[base : base + ga * P].rearrange(
                    "(p q) f -> p (q f)", q=ga
                )
                # alternate the two HWDGE rings (sync / scalar) so data loads
                # use both hardware DGE queues
                dma_eng = nc.sync if a % 2 == 0 else nc.scalar
                dma_eng.dma_start(out=dt32[:, 0 : ga * bc], in_=data_src)
                # convert to bf16, group stride f=bc+1; col bc of each group
                # is memset to 1.0 (counts column)
                dt16 = bpool.tile([P, g * f], bf16, tag="dt16")
                dt16_v = dt16[:, 0 : ga * f].rearrange("p (q f) -> p q f", f=f)
                nc.scalar.activation(
                    out=dt16_v[:, :, 0:bc],
                    in_=dt32[:, 0 : ga * bc].rearrange("p (q f) -> p q f", f=bc),
                    func=mybir.ActivationFunctionType.Copy,
                )
                nc.gpsimd.memset(dt16_v[:, :, bc : bc + 1], 1.0)
                # one-hot for all ga tiles of this group in ONE DVE op:
                # oh[p, q, k] = (iota[p, k] == bins[p, bt+q])
                oh = ohpool.tile([P, g * K], bf16, tag="oh")
                nc.vector.tensor_tensor(
                    out=oh[:, 0 : ga * K].rearrange("p (q k) -> p q k", q=ga),
                    in0=iota_b[:].unsqueeze(1).broadcast_to([P, ga, K]),
                    in1=bins[:, bt : bt + ga].unsqueeze(2).broadcast_to([P, ga, K]),
                    op=iseq,
                )
                for q in range(ga):
                    t = bt + q
                    rhs = dt16[:, q * f : (q + 1) * f]
                    nc.tensor.matmul(
                        ps0[:], oh[:, q * K : q * K + P], rhs,
                        start=(t == 0), stop=(t == nt - 1),
                    )
                    nc.tensor.matmul(
                        ps1[:], oh[:, q * K + P : (q + 1) * K], rhs,
                        start=(t == 0), stop=(t == nt - 1),
                    )
                base += ga * P
                bt += ga

            # local sums -> DRAM bounce -> ReduceScatter (each core gets 16
            # bin-rows of each half) -> divide -> tiny output; host stitches
            # the 8 shards back together. The two halves drain via
            # independent copy+DMA chains on separate engines/rings.
            sums = finp.tile([P, 2 * f], mybir.dt.float32)
            cc_in = dramp.tile([P, 2 * f], mybir.dt.float32)
            cc_out = dramp.tile([nper, 2 * f], mybir.dt.float32)
            nc.vector.tensor_copy(out=sums[:, 0:f], in_=ps0[:])
            nc.sync.dma_start(out=cc_in[:, 0:f], in_=sums[:, 0:f])
            nc.scalar.activation(
                out=sums[:, f : 2 * f], in_=ps1[:],
                func=mybir.ActivationFunctionType.Copy,
            )
            nc.scalar.dma_start(out=cc_in[:, f : 2 * f], in_=sums[:, f : 2 * f])
            nc.gpsimd.collective_compute(
                "ReduceScatter",
                mybir.AluOpType.add,
                replica_groups=[list(range(n_cores))],
                ins=[cc_in[:].opt()],
                outs=[cc_out[:].opt()],
            )
            asum = finp.tile([nper, 2 * f], mybir.dt.float32)
            nc.sync.dma_start(out=asum[:], in_=cc_out[:])

            # means = sums / counts (counts in col f-1 of each half)
            rec = finp.tile([nper, 2], f32)
            nc.vector.reciprocal(out=rec[:, 0:1], in_=asum[:, f - 1 : f])
            nc.vector.reciprocal(out=rec[:, 1:2], in_=asum[:, 2 * f - 1 : 2 * f])
            means = finp.tile([nper, 2 * f], f32)
            nc.vector.tensor_tensor(
                out=means[:, 0:f], in0=asum[:, 0:f],
                in1=rec[:, 0:1].to_broadcast([nper, f]), op=mult,
            )
            nc.vector.tensor_tensor(
                out=means[:, f : 2 * f], in0=asum[:, f : 2 * f],
                in1=rec[:, 1:2].to_broadcast([nper, f]), op=mult,
            )
            nc.sync.dma_start(out=out[:], in_=means[:])

    nc.compile()
    return nc


_NC_CACHE: dict = {}


def _get_nc(nloc, g, bc):
    key = (nloc, g, bc)
    if key not in _NC_CACHE:
        _NC_CACHE[key] = build_nc(nloc, g, bc)
    return _NC_CACHE[key]


def _pack_coord(col, plan):
    """Pack one coordinate column [nloc] -> [P, nt] per the group plan."""
    segs = []
    base = 0
    for ga in plan:
        segs.append(col[base : base + ga * P].reshape(P, ga))
        base += ga * P
    return np.concatenate(segs, axis=1)


def make_in_maps(coords, values, g, n_cores=N_CORES):
    b, n, c = values.shape
    bc = b * c
    nloc = n // n_cores
    plan = group_plan(nloc, g)
    data_t = values.transpose(1, 0, 2).reshape(n, bc)
    iota_np = np.ascontiguousarray(
        np.broadcast_to(np.arange(K, dtype=np.float32), (P, K))
    )
    in_maps = []
    for i in range(n_cores):
        sl = slice(i * nloc, (i + 1) * nloc)
        xs = _pack_coord(coords[sl, 0], plan)
        ys = _pack_coord(coords[sl, 1], plan)
        xyp_np = np.ascontiguousarray(
            np.concatenate([xs, ys], axis=1), dtype=np.float32
        )
        in_maps.append(
            {
                "data": np.ascontiguousarray(data_t[sl]),
                "xyp": xyp_np,
                "iota": iota_np,
            }
        )
    return in_maps


def postprocess(outs, b, c, n_cores=N_CORES):
    """Stitch per-core ReduceScatter shards [nper, 2f] into [B, K*C].

    Core r's rows i are bins r*nper+i (cols 0:f) and 128+r*nper+i (cols
    f:2f).
    """
    bc = b * c
    f = bc + 1
    nper = P // n_cores
    means = np.empty((K, bc), np.float32)
    for r, o in enumerate(outs):
        means[r * nper : (r + 1) * nper] = o[:, 0:bc]
        means[P + r * nper : P + (r + 1) * nper] = o[:, f : f + bc]
    return np.ascontiguousarray(
        means.reshape(K, b, c).transpose(1, 0, 2).reshape(b, K * c)
    ).astype(np.float32)


def kernel(coords, values):
    coords = np.asarray(coords, dtype=np.float32)
    values = np.asarray(values, dtype=np.float32)
    b, n, c = values.shape
    nloc = n // N_CORES
    g = 16
    nc = _get_nc(nloc, g, b * c)
    in_maps = make_in_maps(coords, values, g)
    res = run_bass_kernel_spmd(nc, in_maps, list(range(N_CORES)))
    return postprocess([r["out"] for r in res.results], b, c)


if __name__ == "__main__":
    np.random.seed(0)
    n = 16384
    coords = np.random.uniform(-0.999, 0.999, (n, 2)).astype(np.float32)
    values = np.random.randn(4, n, 64).astype(np.float32)
    got = kernel(coords, values)
    print("out", got.shape, got.dtype, got[:2, :4])


# revision 36
# speedup vs baseline: 1.1634x; 1.1634x over previous
"""Histogram-binning (AdaptiveAvgPoolSequence) Trainium2 kernel.

Full inputs -> shard points N across 8 NeuronCores -> per-core one-hot
matmul accumulation of per-bin sums+counts -> AllReduce -> divide ->
gather output.

Self-contained: hardcodes shapes from the problem spec.
"""

import sys

import numpy as np

sys.path.insert(0, "/opt/trn_rl_repo")

from concourse import bacc, bass, mybir  # noqa: E402
import concourse.tile as tile  # noqa: E402
from concourse.bass_utils import run_bass_kernel_spmd  # noqa: E402

P = 128  # partitions / points per matmul tile
K = 256  # bins (16 x 16)
H = 16
EPS = 1e-6
N_CORES = 8
MAGIC = 12582912.0  # 1.5 * 2**23: float32 round-to-int trick
SCALE = 16.0 / (2.0 + 2.0 * EPS)
OFF = (1.0 + EPS) * SCALE


def group_plan(nloc: int, g: int):
    """DMA group sizes (point-tiles each): g-sized groups with a tapered
    tail so the last group's convert/one-hot/matmul chain is short."""
    nt = nloc // P
    taper = [t for t in (g // 2, g // 4, g // 8, g // 8) if t >= 1]
    while sum(taper) + g > nt:  # tiny problems: no taper
        taper = taper[1:] if taper else []
    body = (nt - sum(taper)) // g
    plan = [g] * body + taper
    rem = nt - sum(plan)
    assert rem == 0, (nt, plan)
    return plan


def build_nc(nloc: int, g: int, bc: int, n_cores: int = N_CORES):
    """Build the per-core Bass graph.

    nloc: points per core; g: point-tiles per DMA group (also the number of
    consecutive DRAM rows each partition loads contiguously); bc: B*C.

    Data layout: group a (size ga) covers points [base, base+ga*P). Partition
    p holds DRAM rows base + p*ga .. + ga (a contiguous ga*bc*4-byte run),
    i.e. point id = base + p*ga + q -> (partition p, sub-tile q). Host-side
    coords packing must use the same order.
    """
    f = bc + 1  # +1 ones column -> counts
    nt = nloc // P
    plan = group_plan(nloc, g)
    assert nloc % P == 0

    nper = P // n_cores  # bins-rows per core after ReduceScatter
    nc = bacc.Bacc(
        "TRN2",
        target_bir_lowering=False,
        debug=False,
        num_devices=n_cores,
    )
    data = nc.declare_dram_parameter("data", [nloc, bc], mybir.dt.float32, isOutput=False)
    xyp = nc.declare_dram_parameter("xyp", [P, 2 * nt], mybir.dt.float32, isOutput=False)
    iota = nc.declare_dram_parameter("iota", [P, K], mybir.dt.float32, isOutput=False)
    out = nc.declare_dram_parameter("out", [nper, 2 * f], mybir.dt.float32, isOutput=True)

    add = mybir.AluOpType.add
    sub = mybir.AluOpType.subtract
    mult = mybir.AluOpType.mult
    iseq = mybir.AluOpType.is_equal
    bf16 = mybir.dt.bfloat16
    f32 = mybir.dt.float32

    with tile.TileContext(nc) as tc:
        with (
            tc.tile_pool(name="const", bufs=1) as constp,
            tc.tile_pool(name="coord", bufs=1) as coordp,
            tc.tile_pool(name="dfp", bufs=6 if g <= 16 else 3) as dpool,
            tc.tile_pool(name="dbf", bufs=4 if g <= 16 else 2) as bpool,
            tc.tile_pool(name="oh", bufs=3 if g <= 16 else 2) as ohpool,
            tc.tile_pool(name="fin", bufs=1) as finp,
            tc.tile_pool(name="ps", bufs=1, space="PSUM") as psump,
            tc.tile_pool(name="dram", bufs=1, space="DRAM") as dramp,
        ):
            # iota constant: row 0..K-1 per partition, f32 + bf16 copies.
            # Small loads ride the gpsimd SWDGE ring so the two HWDGE rings
            # start streaming data immediately.
            iota_f = constp.tile([P, K], f32)
            nc.gpsimd.dma_start(out=iota_f[:], in_=iota[:])
            iota_b = constp.tile([P, K], bf16)
            nc.vector.tensor_copy(out=iota_b[:], in_=iota_f[:])

            # Tiny warm-up collective up front: wakes the ncfw/TOPSP path so
            # the real ReduceScatter at the end skips the ~11.5us cold
            # trigger-to-start delay.
            warm_in = dramp.tile([P, 4], f32)
            warm_out = dramp.tile([nper, 4], f32)
            warm_sb = constp.tile([P, 4], f32)
            nc.vector.tensor_copy(out=warm_sb[:], in_=iota_f[:, 0:4])
            nc.gpsimd.dma_start(out=warm_in[:], in_=warm_sb[:])
            nc.gpsimd.collective_compute(
                "ReduceScatter",
                mybir.AluOpType.add,
                replica_groups=[list(range(n_cores))],
                ins=[warm_in[:].opt()],
                outs=[warm_out[:].opt()],
            )

            # coords -> per-tile bin column. xyp[:, t] = x of point t*P+p,
            # xyp[:, nt+t] = y.
            xy = coordp.tile([P, 2 * nt], f32)
            nc.gpsimd.dma_start(out=xy[:], in_=xyp[:])
            # v = x*SCALE + (OFF - 0.5); k = (v + MAGIC) - MAGIC  (= round(v))
            v = coordp.tile([P, 2 * nt], f32)
            nc.vector.tensor_scalar(
                out=v[:], in0=xy[:], scalar1=SCALE, scalar2=OFF - 0.5, op0=mult, op1=add
            )
            kxy = coordp.tile([P, 2 * nt], f32)
            nc.vector.tensor_scalar(
                out=kxy[:], in0=v[:], scalar1=MAGIC, scalar2=MAGIC, op0=add, op1=sub
            )
            # bins = kx + 16*ky (bf16: exact for integers < 256)
            bins = coordp.tile([P, nt], bf16)
            nc.vector.scalar_tensor_tensor(
                out=bins[:], in0=kxy[:, nt:], scalar=16.0, in1=kxy[:, 0:nt],
                op0=mult, op1=add,
            )

            ps0 = psump.tile([P, f], mybir.dt.float32)
            ps1 = psump.tile([P, f], mybir.dt.float32)

            base = 0  # point offset of current group
            bt = 0  # tile offset of current group
            for a, ga in enumerate(plan):
                dt32 = dpool.tile([P, g * bc], f32, tag="dt32")
                data_src = data[base : base + ga * P].rearrange(
                    "(p q) f -> p (q f)", q=ga
                )
                # alternate the two HWDGE rings (sync / scalar) so data loads
                # use both hardware DGE queues
                dma_eng = nc.sync if a % 2 == 0 else nc.scalar
                dma_eng.dma_start(out=dt32[:, 0 : ga * bc], in_=data_src)
                # convert to bf16, group stride f=bc+1; col bc of each group
                # is memset to 1.0 (counts column)
                dt16 = bpool.tile([P, g * f], bf16, tag="dt16")
                dt16_v = dt16[:, 0 : ga * f].rearrange("p (q f) -> p q f", f=f)
                nc.scalar.activation(
                    out=dt16_v[:, :, 0:bc],
                    in_=dt32[:, 0 : ga * bc].rearrange("p (q f) -> p q f", f=bc),
                    func=mybir.ActivationFunctionType.Copy,
                )
                nc.gpsimd.memset(dt16_v[:, :, bc : bc + 1], 1.0)
                # one-hot for all ga tiles of this group in ONE DVE op:
                # oh[p, q, k] = (iota[p, k] == bins[p, bt+q])
                oh = ohpool.tile([P, g * K], bf16, tag="oh")
                nc.vector.tensor_tensor(
                    out=oh[:, 0 : ga * K].rearrange("p (q k) -> p q k", q=ga),
                    in0=iota_b[:].unsqueeze(1).broadcast_to([P, ga, K]),
                    in1=bins[:, bt : bt + ga].unsqueeze(2).broadcast_to([P, ga, K]),
                    op=iseq,
                )
                for q in range(ga):
                    t = bt + q
                    rhs = dt16[:, q * f : (q + 1) * f]
                    nc.tensor.matmul(
                        ps0[:], oh[:, q * K : q * K + P], rhs,
                        start=(t == 0), stop=(t == nt - 1),
                    )
                    nc.tensor.matmul(
                        ps1[:], oh[:, q * K + P : (q + 1) * K], rhs,
                        start=(t == 0), stop=(t == nt - 1),
                    )
                base += ga * P
                bt += ga

            # local sums -> DRAM bounce -> ReduceScatter (each core gets 16
            # bin-rows of each half) -> divide -> tiny output; host stitches
            # the 8 shards back together. The two halves drain via
            # independent copy+DMA chains on separate engines/rings.
            sums = finp.tile([P, 2 * f], mybir.dt.float32)
            cc_in = dramp.tile([P, 2 * f], mybir.dt.float32)
            cc_out = dramp.tile([nper, 2 * f], mybir.dt.float32)
            nc.vector.tensor_copy(out=sums[:, 0:f], in_=ps0[:])
            nc.sync.dma_start(out=cc_in[:, 0:f], in_=sums[:, 0:f])
            nc.scalar.activation(
                out=sums[:, f : 2 * f], in_=ps1[:],
                func=mybir.ActivationFunctionType.Copy,
            )
            nc.scalar.dma_start(out=cc_in[:, f : 2 * f], in_=sums[:, f : 2 * f])
            nc.gpsimd.collective_compute(
                "ReduceScatter",
                mybir.AluOpType.add,
                replica_groups=[list(range(n_cores))],
                ins=[cc_in[:].opt()],
                outs=[cc_out[:].opt()],
            )
            asum = finp.tile([nper, 2 * f], mybir.dt.float32)
            nc.sync.dma_start(out=asum[:], in_=cc_out[:])

            # means = sums / counts (counts in col f-1 of each half)
            rec = finp.tile([nper, 2], f32)
            nc.vector.reciprocal(out=rec[:, 0:1], in_=asum[:, f - 1 : f])
            nc.vector.reciprocal(out=rec[:, 1:2], in_=asum[:, 2 * f - 1 : 2 * f])
            means = finp.tile([nper, 2 * f], f32)
            nc.vector.tensor_tensor(
                out=means[:, 0:f], in0=asum[:, 0:f],
                in1=rec[:, 0:1].to_broadcast([nper, f]), op=mult,
            )
            nc.vector.tensor_tensor(
                out=means[:, f : 2 * f], in0=asum[:, f : 2 * f],
                in1=rec[:, 1:2].to_broadcast([nper, f]), op=mult,
            )
            nc.sync.dma_start(out=out[:], in_=means[:])

    nc.compile()
    return nc


_NC_CACHE: dict = {}


def _get_nc(nloc, g, bc):
    key = (nloc, g, bc)
    if key not in _NC_CACHE:
        _NC_CACHE[key] = build_nc(nloc, g, bc)
    return _NC_CACHE[key]


def _pack_coord(col, plan):
    """Pack one coordinate column [nloc] -> [P, nt] per the group plan."""
    segs = []
    base = 0
    for ga in plan:
        segs.append(col[base : base + ga * P].reshape(P, ga))
        base += ga * P
    return np.concatenate(segs, axis=1)


def make_in_maps(coords, values, g, n_cores=N_CORES):
    b, n, c = values.shape
    bc = b * c
    nloc = n // n_cores
    plan = group_plan(nloc, g)
    data_t = values.transpose(1, 0, 2).reshape(n, bc)
    iota_np = np.ascontiguousarray(
        np.broadcast_to(np.arange(K, dtype=np.float32), (P, K))
    )
    in_maps = []
    for i in range(n_cores):
        sl = slice(i * nloc, (i + 1) * nloc)
        xs = _pack_coord(coords[sl, 0], plan)
        ys = _pack_coord(coords[sl, 1], plan)
        xyp_np = np.ascontiguousarray(
            np.concatenate([xs, ys], axis=1), dtype=np.float32
        )
        in_maps.append(
            {
                "data": np.ascontiguousarray(data_t[sl]),
                "xyp": xyp_np,
                "iota": iota_np,
            }
        )
    return in_maps


def postprocess(outs, b, c, n_cores=N_CORES):
    """Stitch per-core ReduceScatter shards [nper, 2f] into [B, K*C].

    Core r's rows i are bins r*nper+i (cols 0:f) and 128+r*nper+i (cols
    f:2f).
    """
    bc = b * c
    f = bc + 1
    nper = P // n_cores
    means = np.empty((K, bc), np.float32)
    for r, o in enumerate(outs):
        means[r * nper : (r + 1) * nper] = o[:, 0:bc]
        means[P + r * nper : P + (r + 1) * nper] = o[:, f : f + bc]
    return np.ascontiguousarray(
        means.reshape(K, b, c).transpose(1, 0, 2).reshape(b, K * c)
    ).astype(np.float32)


def kernel(coords, values):
    coords = np.asarray(coords, dtype=np.float32)
    values = np.asarray(values, dtype=np.float32)
    b, n, c = values.shape
    nloc = n // N_CORES
    g = 32
    nc = _get_nc(nloc, g, b * c)
    in_maps = make_in_maps(coords, values, g)
    res = run_bass_kernel_spmd(nc, in_maps, list(range(N_CORES)))
    return postprocess([r["out"] for r in res.results], b, c)


if __name__ == "__main__":
    np.random.seed(0)
    n = 16384
    coords = np.random.uniform(-0.999, 0.999, (n, 2)).astype(np.float32)
    values = np.random.randn(4, n, 64).astype(np.float32)
    got = kernel(coords, values)
    print("out", got.shape, got.dtype, got[:2, :4])
